# revision 16
# baseline (speedup 1.0000x reference)
"""Trainium2 Bass kernel for nn_Codebook (vq_codebook).

v2: fp16/bf16 split-precision distance matmul (3 terms, 1 cyc/row each vs
fp32's 4 cyc/row), half-scale rounding path, fp16 T-table.

Numerics (CPU-validated bit-equivalent argmin vs the jax fp32 reference):
  ref d2 = fl(fl(rowsq - 2G) + cbsq); argmin == argmax of
  u = fl(fl(G_s - 512*rowsq) - 512*cbsq) where G_s = 1024*G, because
  x2/x512/x1024 are exact fp32 binade shifts (identical rounding+ties).
  G_s is computed as T1+T2+T3 accumulated in PSUM fp32:
    T1 = fp16(x) @ fp16(1024c),  T2 = bf16(x - fp16(x)) @ bf16(1024c),
    T3 = bf16(x) @ bf16(1024c - fp16(1024c))
  (bf16 residuals dodge fp16 subnormals; c-side pre-scaled by 1024 so the
  fp16 primary never hits subnormals either). rowsq from fp16 squares
  (tolerance ~1e-2, delta ~2.4e-2 measured safe on this distribution).

Pipeline per core (16 of 128 images):
  precompute: cbt_s (transposed scaled codebook) -> fp16/bf16 splits,
  cbsq*512 broadcast, T16 table (fp16) in DRAM.
  per image: strided x2 load, ACT/DVE dtype splits, 12 fp16/bf16 matmuls
  + 2 rowsq matmuls per image, ACT pass1 (bias=-512*rowsq), GPSIMD
  subtract 512*cbsq, DVE max8 + find_index8, GPSIMD indirect gather of
  T16[bmu], PE fp16 transposes + ACT copies for unpatchify, DMA out.
"""
import math
import sys

if "/opt/trn_rl_repo" not in sys.path:
    sys.path.insert(0, "/opt/trn_rl_repo")

import numpy as np

import concourse.bass as bass
import concourse.mybir as mybir
from concourse.bass import IndirectOffsetOnAxis
from concourse.masks import make_identity
from concourse.tile import TileContext

# ---------------------------------------------------------------------------
# Patch: this walrus build rejects >1 sem wait on the tail Drain; spread the
# waits across single-wait SP nops instead.
import concourse.tile as _tile_mod
from concourse.vector_clock import ScopedClock as _ScopedClock


def _patched_drain_and_barrier(self, tick_clock, wait_clock):
    nc = self.nc
    drain_inst = nc.sync.drain()
    wait_clock.add_sem_waits(
        drain_inst.ins, _ScopedClock({None: tick_clock.global_clock})
    )
    si = drain_inst.ins.sync_info
    waits = list(si.on_wait) if si is not None else []
    if len(waits) > 1:
        si.on_wait = waits[:1]
        for w in waits[1:]:
            nop = nc.sync.nop(nofuse=True)
            nop.ins.sync_info = mybir.SyncInfo(on_wait=[w], on_update=[])

    nc.all_engine_barrier()
    assert self.sems is not None
    popped = nc._tile_sem_poison_stack.pop()
    assert popped is self._sem_poison
    nc.clear_and_free_semaphores(list(self.sems.allocated().values()))
    nc.all_engine_barrier()


_tile_mod.TileContext._drain_and_barrier = _patched_drain_and_barrier

# Generalized: any instruction may carry at most MAX_WAITS sem waits in this
# walrus build. Hoist extras onto same-engine NoOps committed just before
# (engines execute in order, so an earlier wait is equivalent).
MAX_WAITS = 1
_orig_commit = _tile_mod.TileContext._commit_instruction
_waitsplit_id = [0]


def _patched_commit(self, inst, lazy_reg_writes=True):
    si = inst.sync_info
    if si is not None:
        waits = list(si.on_wait)
        if len(waits) > MAX_WAITS:
            keep = waits[:MAX_WAITS - 1] if MAX_WAITS > 1 else []
            extra = waits[len(keep):]
            si.on_wait = keep + extra[-1:]
            extra = extra[:-1]
            for i in range(0, len(extra), MAX_WAITS):
                _waitsplit_id[0] += 1
                nop = mybir.InstNoOp(
                    name=f"I-waitsplit-{_waitsplit_id[0]}", ins=[], outs=[]
                )
                nop.engine = inst.engine
                nop.sync_info = mybir.SyncInfo(
                    on_wait=extra[i:i + MAX_WAITS], on_update=[]
                )
                self._add_instruction(nop)
    return _orig_commit(self, inst, lazy_reg_writes)


_tile_mod.TileContext._commit_instruction = _patched_commit
# ---------------------------------------------------------------------------

F32 = mybir.dt.float32
F16 = mybir.dt.float16
BF16 = mybir.dt.bfloat16
U32 = mybir.dt.uint32
AX = mybir.AxisListType
ALU = mybir.AluOpType
ACTF = mybir.ActivationFunctionType

N_CORES = 8
NI = 16          # images per core
K = 1024         # codebook entries
D = 256          # embedding dim
NEG_INV_2VAR = -1.0 / (2.0 * (-(256.0 / (2.0 * math.log(0.1)))))
S = 1024.0       # global c-side scale (exact power of 2)


def build_kernel():
    nc = bass.Bass()
    x = nc.dram_tensor("x", [NI, 16, 64, 64], F32, kind="ExternalInput")
    cb = nc.dram_tensor("codebook", [K, D], F32, kind="ExternalInput")
    out = nc.dram_tensor("out", [NI, 16, 64, 64], F32, kind="ExternalOutput")
    t_dram = nc.dram_tensor("t_scratch16", [K, D], F16, kind="Internal")

    with TileContext(nc) as tc:
        with (
            tc.tile_pool(name="const", bufs=1) as cpool,
            tc.tile_pool(name="x2", bufs=4) as xpool,
            tc.tile_pool(name="conv", bufs=3) as vpool,
            tc.tile_pool(name="scores", bufs=3) as spool,
            tc.tile_pool(name="small", bufs=4) as smpool,
            tc.tile_pool(name="q", bufs=12) as qpool,
            tc.tile_pool(name="idx", bufs=33) as ixpool,
            tc.tile_pool(name="outsb", bufs=4) as opool,
            tc.tile_pool(name="pg", bufs=3, space="PSUM") as pg,
            tc.tile_pool(name="pq2", bufs=1, space="PSUM") as pq2,
            tc.tile_pool(name="pu16", bufs=1, space="PSUM") as pu16,
        ):
            # ---------------- constants ----------------
            ident = cpool.tile([128, 128], F32)
            make_identity(nc, ident[:])
            ident16 = cpool.tile([128, 128], F16)
            nc.vector.tensor_copy(ident16[:], ident[:])
            ones_col = cpool.tile([128, 1], F32)
            nc.vector.memset(ones_col[:], 1.0)
            ones_row = cpool.tile([1, 128], F32)
            nc.vector.memset(ones_row[:], 1.0)
            ones16 = cpool.tile([128, 1], F16)
            nc.vector.memset(ones16[:], 1.0)
            ones8 = cpool.tile([128, 8], F32)
            nc.vector.memset(ones8[:], 1.0)

            # W band table [128, 384]: W[p, u] = gauss(u - 128 - p)
            wtab = cpool.tile([128, 384], F32)
            nc.gpsimd.iota(
                wtab[:], pattern=[[1, 384]], base=-128, channel_multiplier=-1,
                allow_small_or_imprecise_dtypes=True,
            )
            wsq = cpool.tile([128, 384], F32)
            nc.scalar.activation(wsq[:], wtab[:], ACTF.Square)
            nc.scalar.activation(wtab[:], wsq[:], ACTF.Exp, scale=NEG_INV_2VAR)
            wtab16 = cpool.tile([128, 384], F16)
            nc.scalar.activation(wtab16[:], wtab[:], ACTF.Copy)

            # codebook natural chunks: cb_all[p, jc*256+d] = cb[jc*128+p, d]
            cb_all = cpool.tile([128, 8 * D], F32)
            nc.sync.dma_start(
                out=cb_all[:].rearrange("p (jc d) -> p jc d", jc=8),
                in_=cb[:].rearrange("(jc p) d -> p jc d", p=128),
            )
            # fp16 scaled copy for the T-table matmuls (descaled at T copy)
            cb16_all = cpool.tile([128, 8 * D], F16)
            nc.scalar.activation(cb16_all[:], cb_all[:], ACTF.Copy, scale=S)

            # cbt_s [128, 1024] x2: scaled transposed codebook
            # p<64 -> S*cb[j, 4a+k]; p>=64 -> S*cb[j, 4a+k+1]
            cbt0 = cpool.tile([128, K], F32, tag="cbt0")
            cbt2 = cpool.tile([128, K], F32, tag="cbt2")
            cbt = {0: cbt0, 2: cbt2}
            stage0 = cpool.tile([64, K], F32, tag="stage0")
            stage2 = cpool.tile([64, K], F32, tag="stage2")
            stage = {0: stage0, 2: stage2}
            cb_r = cb_all[:].rearrange("p (jc a pw) -> p jc a pw", jc=8, pw=4)
            for jc in range(8):
                ptp = pg.tile([128, 1024], F32, tag="pg", name="ptp")
                for pw in range(4):
                    k = 0 if pw < 2 else 2
                    tgt = cbt[k] if pw % 2 == 0 else stage[k]
                    tp = ptp[0:64, pw * 128:(pw + 1) * 128]
                    nc.tensor.transpose(out=tp, in_=cb_r[:, jc, :, pw],
                                        identity=ident[:])
                    nc.scalar.activation(
                        tgt[0:64, jc * 128:(jc + 1) * 128], tp, ACTF.Copy,
                        scale=S,
                    )
            for k in (0, 2):
                nc.sync.dma_start(out=cbt[k][64:128, :], in_=stage[k][0:64, :])

            # fp16 primary + bf16 residual splits of cbt_s
            chs = {k: cpool.tile([128, K], F16, tag=f"chs{k}",
                                 name=f"chs{k}") for k in (0, 2)}
            chb = {k: cpool.tile([128, K], BF16, tag=f"chb{k}",
                                 name=f"chb{k}") for k in (0, 2)}
            clb = {k: cpool.tile([128, K], BF16, tag=f"clb{k}",
                                 name=f"clb{k}") for k in (0, 2)}
            for k in (0, 2):
                nc.scalar.activation(chs[k][:], cbt[k][:], ACTF.Copy)
                nc.scalar.activation(chb[k][:], cbt[k][:], ACTF.Copy)
                nc.vector.tensor_tensor(
                    clb[k][:], cbt[k][:], chs[k][:], ALU.subtract
                )

            # cbsq_x broadcast [128, 1024] = 512 * cbsq  (from scaled cbt:
            # sum((S*c)^2) * 2^-11 == 512 * cbsq bitwise)
            cbsq_bcast = cpool.tile([128, K], F32)
            sqtmp = spool.tile([128, K], F32, tag="scores", bufs=3)
            pcb = pg.tile([128, 1024], F32, tag="pg")
            pbs = (pcb[0:1, 0:512], pcb[0:1, 512:1024])
            for ki, k in enumerate((0, 2)):
                nc.vector.tensor_tensor(sqtmp[:], cbt[k][:], cbt[k][:], ALU.mult)
                for h in range(2):
                    nc.tensor.matmul(
                        pbs[h], ones_col[:], sqtmp[:, h * 512:(h + 1) * 512],
                        start=(ki == 0), stop=(ki == 1),
                    )
            cbsq_row = smpool.tile([1, K], F32, tag="cbsqrow", bufs=1)
            for h in range(2):
                nc.scalar.activation(
                    cbsq_row[0:1, h * 512:(h + 1) * 512], pbs[h], ACTF.Copy
                )
            pc = pg.tile([128, 1024], F32, tag="pg")
            for h in range(2):
                nc.tensor.matmul(
                    pc[:, h * 512:(h + 1) * 512], ones_row[:],
                    cbsq_row[0:1, h * 512:(h + 1) * 512],
                    start=True, stop=True,
                )
            nc.scalar.activation(cbsq_bcast[:], pc[:], ACTF.Copy, scale=2.0**-11)

            # T16 table: T[bc*128+p, :] = sum_j gauss(b-j) cb[j, :] in fp16
            t_write_insts = []
            for bc in range(8):
                pt = pg.tile([128, 1024], F32, tag="pg")
                deltas = [d_ for d_ in (-1, 0, 1) if 0 <= bc + d_ < 8]
                for i, d_ in enumerate(deltas):
                    off = 128 * (1 - d_)
                    jc = bc + d_
                    nc.tensor.matmul(
                        pt[:, 0:D],
                        wtab16[:, off:off + 128],
                        cb16_all[:, jc * D:(jc + 1) * D],
                        start=(i == 0), stop=(i == len(deltas) - 1),
                    )
                t_sb = smpool.tile([128, D], F16, tag="tsb", bufs=3)
                nc.scalar.activation(t_sb[:], pt[:, 0:D], ACTF.Copy,
                                     scale=1.0 / S)
                wi = nc.sync.dma_start(
                    out=t_dram[bc * 128:(bc + 1) * 128, :], in_=t_sb[:]
                )
                t_write_insts.append(wi.ins)

            # ---------------- main loop ----------------
            # Software-pipelined: iteration n emits loads+conversions for
            # image n, distance+scores for image n-1, unpatchify for n-3.
            idx_tiles = {}
            rowsq_neg = {}
            GS = 640  # columns 0:GS subtracted on GPSIMD, GS: on DVE

            def emit_load_convert(n):
                x2 = xpool.tile([128, 1028], F32, name="x2t")
                for ph in range(4):
                    nc.sync.dma_start(
                        out=x2[64 + ph:128:4, 0:1024],
                        in_=x[n][:, ph::4, :],
                    )
                nc.sync.dma_start(out=x2[0:64, 1:1025], in_=x2[64:128, 0:1024])
                # odd columns only (all matmul operands live on odd cols)
                x2odd = x2[:, 1:1025].rearrange(
                    "p (i two) -> p i two", two=2)[:, :, 0]
                x2h = vpool.tile([128, 514], F16, tag="x2h", name="x2h")
                nc.scalar.activation(x2h[:, 0:512], x2odd, ACTF.Copy)
                x2l = vpool.tile([128, 514], BF16, tag="x2l", name="x2l")
                nc.gpsimd.tensor_tensor(x2l[:, 0:512], x2odd, x2h[:, 0:512],
                                        ALU.subtract)
                x2sq = vpool.tile([128, 514], F16, tag="x2sq",
                                  name="x2sq")
                nc.scalar.activation(x2sq[:, 0:512], x2odd, ACTF.Square)
                return x2h, x2l, x2sq

            def emit_rowsq(n, x2sq):
                # rowsq*(-512) per row, directly in [128,1] layout:
                # lhsT = x2sq slices (odd-col layout, stride 2), rhs = ones16
                for t in range(2):
                    prt = pq2.tile([128, 1], F32, tag="purq", name="prt")
                    for ki, k in enumerate((0, 2)):
                        j0 = (k >> 1) + 256 * t
                        lhsT = x2sq[:, j0:j0 + 256].rearrange(
                            "p (i two) -> p i two", two=2)[:, :, 0]
                        nc.tensor.matmul(
                            prt[:], lhsT, ones16[:],
                            start=(ki == 0), stop=(ki == 1),
                        )
                    rq = smpool.tile([128, 1], F32, tag="rowsqneg", bufs=4,
                                     name="rq")
                    nc.scalar.activation(rq[:], prt[:], ACTF.Copy,
                                         scale=-512.0)
                    rowsq_neg[(n, t)] = rq

            def emit_scores(n, x2h, x2l):
                for t in range(2):
                    pgt = pg.tile([128, 1024], F32, tag="pg",
                                  name="pgt")
                    gi = 0
                    for (xt, ct) in ((x2h, chs), (x2l, chb), (x2h, clb)):
                        for k in (0, 2):
                            j0 = (k >> 1) + 256 * t
                            lhsT = xt[:, j0:j0 + 256].rearrange(
                                "p (i two) -> p i two", two=2)[:, :, 0]
                            st, sp = (gi == 0), (gi == 5)
                            for h in range(2):
                                nc.tensor.matmul(
                                    pgt[:, h * 512:(h + 1) * 512],
                                    lhsT, ct[k][:, h * 512:(h + 1) * 512],
                                    start=st, stop=sp,
                                )
                            gi += 1
                    # pass1 on ACT: a1 = fl(pgt - 512*rowsq)
                    sc = spool.tile([128, K], F32, tag="scores",
                                    name="sc")
                    nc.scalar.activation(
                        sc[:], pgt[:], ACTF.Identity,
                        bias=rowsq_neg[(n, t)][:, 0:1], scale=1.0,
                    )
                    # pass2 split: u = fl(a1 - 512*cbsq)
                    nc.gpsimd.tensor_tensor(
                        sc[:, 0:GS], sc[:, 0:GS], cbsq_bcast[:, 0:GS],
                        ALU.subtract
                    )
                    nc.vector.tensor_tensor(
                        sc[:, GS:K], sc[:, GS:K], cbsq_bcast[:, GS:K],
                        ALU.subtract
                    )
                    mx8 = smpool.tile([128, 8], F32, tag="mx8",
                                      name="mx8")
                    idx8 = ixpool.tile([128, 8], U32, tag="idx8",
                                       name="idx8")
                    nc.vector.max(mx8[:], sc[:])
                    nc.vector.max_index(idx8[:], mx8[:], sc[:])
                    idx_tiles[(n, t)] = idx8
                    # gather interleaved on the GPSIMD queue
                    q = qpool.tile([128, D], F16, tag="q", name="qt")
                    gri = nc.gpsimd.indirect_dma_start(
                        out=q[:],
                        out_offset=None,
                        in_=t_dram[:],
                        in_offset=IndirectOffsetOnAxis(
                            ap=idx8[:, 0:1], axis=0),
                    )
                    for twi in t_write_insts:
                        _tile_mod.add_dep_helper(
                            gri.ins, twi, reason="gather waits for T table"
                        )
                    q_tiles[(n, t)] = q

            q_tiles = {}

            def emit_unpat(n):
                out_sb = opool.tile([64, 1024], F32, name="osb")
                for t in range(2):
                    q = q_tiles.pop((n, t))
                    q_r = q[:].rearrange("p (a pw) -> p a pw", pw=4)
                    put = pu16.tile([64, 512], F16, tag="pu16",
                                    name="put")
                    for pw in range(4):
                        nc.tensor.transpose(
                            out=put[:, pw * 128:(pw + 1) * 128],
                            in_=q_r[:, :, pw], identity=ident16[:],
                        )
                    o_r = out_sb[:].rearrange(
                        "p (hp wp pw) -> p pw hp wp", wp=16, pw=4
                    )
                    nc.scalar.activation(
                        o_r[0:64, :, 8 * t:8 * (t + 1), :], put[:], ACTF.Copy
                    )
                for ph in range(4):
                    eng = nc.scalar if ph == 0 else nc.sync
                    eng.dma_start(
                        out=out[n][:, ph::4, :],
                        in_=out_sb[ph:64:4, :],
                    )

            conv = {}
            conv[0] = emit_load_convert(0)
            emit_rowsq(0, conv[0][2])
            conv[1] = emit_load_convert(1)
            for n in range(NI):
                if n + 2 < NI:
                    conv[n + 2] = emit_load_convert(n + 2)
                x2h, x2l, _ = conv.pop(n)
                emit_scores(n, x2h, x2l)
                if n + 1 < NI:
                    emit_rowsq(n + 1, conv[n + 1][2])
                if n >= 5:
                    emit_unpat(n - 5)
            for m in range(NI - 5, NI):
                emit_unpat(m)

    return nc
_NC_CACHE = None


def _get_nc():
    global _NC_CACHE
    if _NC_CACHE is None:
        _NC_CACHE = build_kernel()
    return _NC_CACHE


def kernel(**inputs: np.ndarray) -> np.ndarray:
    from concourse.bass_utils import run_bass_kernel_spmd

    x = np.ascontiguousarray(inputs["x"], dtype=np.float32)
    cb = np.ascontiguousarray(inputs["codebook"], dtype=np.float32)
    assert x.shape == (128, 16, 64, 64) and cb.shape == (K, D)

    nc = _get_nc()
    in_maps = [
        {"x": x[i * NI:(i + 1) * NI], "codebook": cb} for i in range(N_CORES)
    ]
    res = run_bass_kernel_spmd(nc, in_maps, core_ids=list(range(N_CORES)))
    return np.concatenate([res.results[i]["out"] for i in range(N_CORES)], axis=0)


# revision 17
# speedup vs baseline: 1.0889x; 1.0889x over previous
"""Trainium2 Bass kernel for nn_Codebook (vq_codebook).

v2: fp16/bf16 split-precision distance matmul (3 terms, 1 cyc/row each vs
fp32's 4 cyc/row), half-scale rounding path, fp16 T-table.

Numerics (CPU-validated bit-equivalent argmin vs the jax fp32 reference):
  ref d2 = fl(fl(rowsq - 2G) + cbsq); argmin == argmax of
  u = fl(fl(G_s - 512*rowsq) - 512*cbsq) where G_s = 1024*G, because
  x2/x512/x1024 are exact fp32 binade shifts (identical rounding+ties).
  G_s is computed as T1+T2+T3 accumulated in PSUM fp32:
    T1 = fp16(x) @ fp16(1024c),  T2 = bf16(x - fp16(x)) @ bf16(1024c),
    T3 = bf16(x) @ bf16(1024c - fp16(1024c))
  (bf16 residuals dodge fp16 subnormals; c-side pre-scaled by 1024 so the
  fp16 primary never hits subnormals either). rowsq from fp16 squares
  (tolerance ~1e-2, delta ~2.4e-2 measured safe on this distribution).

Pipeline per core (16 of 128 images):
  precompute: cbt_s (transposed scaled codebook) -> fp16/bf16 splits,
  cbsq*512 broadcast, T16 table (fp16) in DRAM.
  per image: strided x2 load, ACT/DVE dtype splits, 12 fp16/bf16 matmuls
  + 2 rowsq matmuls per image, ACT pass1 (bias=-512*rowsq), GPSIMD
  subtract 512*cbsq, DVE max8 + find_index8, GPSIMD indirect gather of
  T16[bmu], PE fp16 transposes + ACT copies for unpatchify, DMA out.
"""
import math
import sys

if "/opt/trn_rl_repo" not in sys.path:
    sys.path.insert(0, "/opt/trn_rl_repo")

import numpy as np

import concourse.bass as bass
import concourse.mybir as mybir
from concourse.bass import IndirectOffsetOnAxis
from concourse.masks import make_identity
from concourse.tile import TileContext

# ---------------------------------------------------------------------------
# Patch: this walrus build rejects >1 sem wait on the tail Drain; spread the
# waits across single-wait SP nops instead.
import concourse.tile as _tile_mod
from concourse.vector_clock import ScopedClock as _ScopedClock


def _patched_drain_and_barrier(self, tick_clock, wait_clock):
    nc = self.nc
    drain_inst = nc.sync.drain()
    wait_clock.add_sem_waits(
        drain_inst.ins, _ScopedClock({None: tick_clock.global_clock})
    )
    si = drain_inst.ins.sync_info
    waits = list(si.on_wait) if si is not None else []
    if len(waits) > 1:
        si.on_wait = waits[:1]
        for w in waits[1:]:
            nop = nc.sync.nop(nofuse=True)
            nop.ins.sync_info = mybir.SyncInfo(on_wait=[w], on_update=[])

    nc.all_engine_barrier()
    assert self.sems is not None
    popped = nc._tile_sem_poison_stack.pop()
    assert popped is self._sem_poison
    nc.clear_and_free_semaphores(list(self.sems.allocated().values()))
    nc.all_engine_barrier()


_tile_mod.TileContext._drain_and_barrier = _patched_drain_and_barrier

# Generalized: any instruction may carry at most MAX_WAITS sem waits in this
# walrus build. Hoist extras onto same-engine NoOps committed just before
# (engines execute in order, so an earlier wait is equivalent).
MAX_WAITS = 1
_orig_commit = _tile_mod.TileContext._commit_instruction
_waitsplit_id = [0]


def _patched_commit(self, inst, lazy_reg_writes=True):
    si = inst.sync_info
    if si is not None:
        waits = list(si.on_wait)
        if len(waits) > MAX_WAITS:
            keep = waits[:MAX_WAITS - 1] if MAX_WAITS > 1 else []
            extra = waits[len(keep):]
            si.on_wait = keep + extra[-1:]
            extra = extra[:-1]
            for i in range(0, len(extra), MAX_WAITS):
                _waitsplit_id[0] += 1
                nop = mybir.InstNoOp(
                    name=f"I-waitsplit-{_waitsplit_id[0]}", ins=[], outs=[]
                )
                nop.engine = inst.engine
                nop.sync_info = mybir.SyncInfo(
                    on_wait=extra[i:i + MAX_WAITS], on_update=[]
                )
                self._add_instruction(nop)
    return _orig_commit(self, inst, lazy_reg_writes)


_tile_mod.TileContext._commit_instruction = _patched_commit
# ---------------------------------------------------------------------------

F32 = mybir.dt.float32
F16 = mybir.dt.float16
BF16 = mybir.dt.bfloat16
U32 = mybir.dt.uint32
AX = mybir.AxisListType
ALU = mybir.AluOpType
ACTF = mybir.ActivationFunctionType

N_CORES = 8
NI = 16          # images per core
K = 1024         # codebook entries
D = 256          # embedding dim
NEG_INV_2VAR = -1.0 / (2.0 * (-(256.0 / (2.0 * math.log(0.1)))))
S = 1024.0       # global c-side scale (exact power of 2)


def build_kernel():
    nc = bass.Bass()
    x = nc.dram_tensor("x", [NI, 16, 64, 64], F32, kind="ExternalInput")
    cb = nc.dram_tensor("codebook", [K, D], F32, kind="ExternalInput")
    out = nc.dram_tensor("out", [NI, 16, 64, 64], F32, kind="ExternalOutput")
    t_dram = nc.dram_tensor("t_scratch16", [K, D], F16, kind="Internal")

    with TileContext(nc) as tc:
        with (
            tc.tile_pool(name="const", bufs=1) as cpool,
            tc.tile_pool(name="x2", bufs=4) as xpool,
            tc.tile_pool(name="conv", bufs=3) as vpool,
            tc.tile_pool(name="scores", bufs=3) as spool,
            tc.tile_pool(name="small", bufs=4) as smpool,
            tc.tile_pool(name="q", bufs=12) as qpool,
            tc.tile_pool(name="idx", bufs=33) as ixpool,
            tc.tile_pool(name="outsb", bufs=4) as opool,
            tc.tile_pool(name="pg", bufs=3, space="PSUM") as pg,
            tc.tile_pool(name="pq2", bufs=1, space="PSUM") as pq2,
            tc.tile_pool(name="pu16", bufs=1, space="PSUM") as pu16,
        ):
            # ---------------- constants ----------------
            ident = cpool.tile([128, 128], F32)
            make_identity(nc, ident[:])
            ident16 = cpool.tile([128, 128], F16)
            nc.vector.tensor_copy(ident16[:], ident[:])
            ones_col = cpool.tile([128, 1], F32)
            nc.vector.memset(ones_col[:], 1.0)
            ones_row = cpool.tile([1, 128], F32)
            nc.vector.memset(ones_row[:], 1.0)
            ones16 = cpool.tile([128, 1], F16)
            nc.vector.memset(ones16[:], 1.0)
            ones8 = cpool.tile([128, 8], F32)
            nc.vector.memset(ones8[:], 1.0)

            # W band table [128, 384]: W[p, u] = gauss(u - 128 - p)
            wtab = cpool.tile([128, 384], F32)
            nc.gpsimd.iota(
                wtab[:], pattern=[[1, 384]], base=-128, channel_multiplier=-1,
                allow_small_or_imprecise_dtypes=True,
            )
            wsq = cpool.tile([128, 384], F32)
            nc.scalar.activation(wsq[:], wtab[:], ACTF.Square)
            nc.scalar.activation(wtab[:], wsq[:], ACTF.Exp, scale=NEG_INV_2VAR)
            wtab16 = cpool.tile([128, 384], F16)
            nc.scalar.activation(wtab16[:], wtab[:], ACTF.Copy)

            # codebook natural chunks: cb_all[p, jc*256+d] = cb[jc*128+p, d]
            cb_all = cpool.tile([128, 8 * D], F32)
            nc.sync.dma_start(
                out=cb_all[:].rearrange("p (jc d) -> p jc d", jc=8),
                in_=cb[:].rearrange("(jc p) d -> p jc d", p=128),
            )
            # fp16 scaled copy for the T-table matmuls (descaled at T copy)
            cb16_all = cpool.tile([128, 8 * D], F16)
            nc.scalar.activation(cb16_all[:], cb_all[:], ACTF.Copy, scale=S)

            # cbt_s [128, 1024] x2: scaled transposed codebook
            # p<64 -> S*cb[j, 4a+k]; p>=64 -> S*cb[j, 4a+k+1]
            cbt0 = cpool.tile([128, K], F32, tag="cbt0")
            cbt2 = cpool.tile([128, K], F32, tag="cbt2")
            cbt = {0: cbt0, 2: cbt2}
            stage0 = cpool.tile([64, K], F32, tag="stage0")
            stage2 = cpool.tile([64, K], F32, tag="stage2")
            stage = {0: stage0, 2: stage2}
            cb_r = cb_all[:].rearrange("p (jc a pw) -> p jc a pw", jc=8, pw=4)
            for jc in range(8):
                ptp = pg.tile([128, 1024], F32, tag="pg", name="ptp")
                for pw in range(4):
                    k = 0 if pw < 2 else 2
                    tgt = cbt[k] if pw % 2 == 0 else stage[k]
                    tp = ptp[0:64, pw * 128:(pw + 1) * 128]
                    nc.tensor.transpose(out=tp, in_=cb_r[:, jc, :, pw],
                                        identity=ident[:])
                    nc.scalar.activation(
                        tgt[0:64, jc * 128:(jc + 1) * 128], tp, ACTF.Copy,
                        scale=S,
                    )
            for k in (0, 2):
                nc.sync.dma_start(out=cbt[k][64:128, :], in_=stage[k][0:64, :])

            # fp16 primary + bf16 residual splits of cbt_s
            chs = {k: cpool.tile([128, K], F16, tag=f"chs{k}",
                                 name=f"chs{k}") for k in (0, 2)}
            chb = {k: cpool.tile([128, K], BF16, tag=f"chb{k}",
                                 name=f"chb{k}") for k in (0, 2)}
            clb = {k: cpool.tile([128, K], BF16, tag=f"clb{k}",
                                 name=f"clb{k}") for k in (0, 2)}
            for k in (0, 2):
                nc.scalar.activation(chs[k][:], cbt[k][:], ACTF.Copy)
                nc.scalar.activation(chb[k][:], cbt[k][:], ACTF.Copy)
                nc.vector.tensor_tensor(
                    clb[k][:], cbt[k][:], chs[k][:], ALU.subtract
                )

            # cbsq_x broadcast [128, 1024] = 512 * cbsq  (from scaled cbt:
            # sum((S*c)^2) * 2^-11 == 512 * cbsq bitwise)
            cbsq_bcast = cpool.tile([128, K], F32)
            sqtmp = spool.tile([128, K], F32, tag="scores", bufs=3)
            pcb = pg.tile([128, 1024], F32, tag="pg")
            pbs = (pcb[0:1, 0:512], pcb[0:1, 512:1024])
            for ki, k in enumerate((0, 2)):
                nc.vector.tensor_tensor(sqtmp[:], cbt[k][:], cbt[k][:], ALU.mult)
                for h in range(2):
                    nc.tensor.matmul(
                        pbs[h], ones_col[:], sqtmp[:, h * 512:(h + 1) * 512],
                        start=(ki == 0), stop=(ki == 1),
                    )
            cbsq_row = smpool.tile([1, K], F32, tag="cbsqrow", bufs=1)
            for h in range(2):
                nc.scalar.activation(
                    cbsq_row[0:1, h * 512:(h + 1) * 512], pbs[h], ACTF.Copy
                )
            pc = pg.tile([128, 1024], F32, tag="pg")
            for h in range(2):
                nc.tensor.matmul(
                    pc[:, h * 512:(h + 1) * 512], ones_row[:],
                    cbsq_row[0:1, h * 512:(h + 1) * 512],
                    start=True, stop=True,
                )
            nc.scalar.activation(cbsq_bcast[:], pc[:], ACTF.Copy, scale=2.0**-11)

            # T16 table: T[bc*128+p, :] = sum_j gauss(b-j) cb[j, :] in fp16
            t_write_insts = []
            for bc in range(8):
                pt = pg.tile([128, 1024], F32, tag="pg")
                deltas = [d_ for d_ in (-1, 0, 1) if 0 <= bc + d_ < 8]
                for i, d_ in enumerate(deltas):
                    off = 128 * (1 - d_)
                    jc = bc + d_
                    nc.tensor.matmul(
                        pt[:, 0:D],
                        wtab16[:, off:off + 128],
                        cb16_all[:, jc * D:(jc + 1) * D],
                        start=(i == 0), stop=(i == len(deltas) - 1),
                    )
                t_sb = smpool.tile([128, D], F16, tag="tsb", bufs=3)
                nc.scalar.activation(t_sb[:], pt[:, 0:D], ACTF.Copy,
                                     scale=1.0 / S)
                wi = nc.sync.dma_start(
                    out=t_dram[bc * 128:(bc + 1) * 128, :], in_=t_sb[:]
                )
                t_write_insts.append(wi.ins)

            # ---------------- main loop ----------------
            # Software-pipelined: iteration n emits loads+conversions for
            # image n, distance+scores for image n-1, unpatchify for n-3.
            idx_tiles = {}
            rowsq_neg = {}
            GS = 768  # columns 0:GS subtracted on GPSIMD, GS: on DVE

            def emit_load_convert(n):
                x2 = xpool.tile([128, 1028], F32, name="x2t")
                for ph in range(4):
                    nc.sync.dma_start(
                        out=x2[64 + ph:128:4, 0:1024],
                        in_=x[n][:, ph::4, :],
                    )
                nc.sync.dma_start(out=x2[0:64, 1:1025], in_=x2[64:128, 0:1024])
                # odd columns only (all matmul operands live on odd cols)
                x2odd = x2[:, 1:1025].rearrange(
                    "p (i two) -> p i two", two=2)[:, :, 0]
                x2h = vpool.tile([128, 514], F16, tag="x2h", name="x2h")
                nc.scalar.activation(x2h[:, 0:512], x2odd, ACTF.Copy)
                x2l = vpool.tile([128, 514], BF16, tag="x2l", name="x2l")
                nc.vector.tensor_tensor(x2l[:, 0:512], x2odd, x2h[:, 0:512],
                                        ALU.subtract)
                x2sq = vpool.tile([128, 514], F16, tag="x2sq",
                                  name="x2sq")
                nc.scalar.activation(x2sq[:, 0:512], x2odd, ACTF.Square)
                return x2h, x2l, x2sq

            def emit_rowsq(n, x2sq):
                # rowsq*(-512) per row, directly in [128,1] layout:
                # lhsT = x2sq slices (odd-col layout, stride 2), rhs = ones16
                for t in range(2):
                    prt = pq2.tile([128, 1], F32, tag="purq", name="prt")
                    for ki, k in enumerate((0, 2)):
                        j0 = (k >> 1) + 256 * t
                        lhsT = x2sq[:, j0:j0 + 256].rearrange(
                            "p (i two) -> p i two", two=2)[:, :, 0]
                        nc.tensor.matmul(
                            prt[:], lhsT, ones16[:],
                            start=(ki == 0), stop=(ki == 1),
                        )
                    rq = smpool.tile([128, 1], F32, tag="rowsqneg", bufs=4,
                                     name="rq")
                    nc.scalar.activation(rq[:], prt[:], ACTF.Copy,
                                         scale=-512.0)
                    rowsq_neg[(n, t)] = rq

            def emit_scores(n, x2h, x2l):
                for t in range(2):
                    pgt = pg.tile([128, 1024], F32, tag="pg",
                                  name="pgt")
                    gi = 0
                    for (xt, ct) in ((x2h, chs), (x2l, chb), (x2h, clb)):
                        for k in (0, 2):
                            j0 = (k >> 1) + 256 * t
                            lhsT = xt[:, j0:j0 + 256].rearrange(
                                "p (i two) -> p i two", two=2)[:, :, 0]
                            st, sp = (gi == 0), (gi == 5)
                            for h in range(2):
                                nc.tensor.matmul(
                                    pgt[:, h * 512:(h + 1) * 512],
                                    lhsT, ct[k][:, h * 512:(h + 1) * 512],
                                    start=st, stop=sp,
                                )
                            gi += 1
                    # pass1 on ACT: a1 = fl(pgt - 512*rowsq)
                    sc = spool.tile([128, K], F32, tag="scores",
                                    name="sc")
                    nc.scalar.activation(
                        sc[:], pgt[:], ACTF.Identity,
                        bias=rowsq_neg[(n, t)][:, 0:1], scale=1.0,
                    )
                    # pass2 split: u = fl(a1 - 512*cbsq)
                    nc.gpsimd.tensor_tensor(
                        sc[:, 0:GS], sc[:, 0:GS], cbsq_bcast[:, 0:GS],
                        ALU.subtract
                    )
                    nc.vector.tensor_tensor(
                        sc[:, GS:K], sc[:, GS:K], cbsq_bcast[:, GS:K],
                        ALU.subtract
                    )
                    mx8 = smpool.tile([128, 8], F32, tag="mx8",
                                      name="mx8")
                    idx8 = ixpool.tile([128, 8], U32, tag="idx8",
                                       name="idx8")
                    nc.vector.max(mx8[:], sc[:])
                    nc.vector.max_index(idx8[:], mx8[:], sc[:])
                    idx_tiles[(n, t)] = idx8
                    # gather interleaved on the GPSIMD queue
                    q = qpool.tile([128, D], F16, tag="q", name="qt")
                    gri = nc.gpsimd.indirect_dma_start(
                        out=q[:],
                        out_offset=None,
                        in_=t_dram[:],
                        in_offset=IndirectOffsetOnAxis(
                            ap=idx8[:, 0:1], axis=0),
                    )
                    for twi in t_write_insts:
                        _tile_mod.add_dep_helper(
                            gri.ins, twi, reason="gather waits for T table"
                        )
                    q_tiles[(n, t)] = q

            q_tiles = {}

            def emit_unpat(n):
                out_sb = opool.tile([64, 1024], F32, name="osb")
                for t in range(2):
                    q = q_tiles.pop((n, t))
                    q_r = q[:].rearrange("p (a pw) -> p a pw", pw=4)
                    put = pu16.tile([64, 512], F16, tag="pu16",
                                    name="put")
                    for pw in range(4):
                        nc.tensor.transpose(
                            out=put[:, pw * 128:(pw + 1) * 128],
                            in_=q_r[:, :, pw], identity=ident16[:],
                        )
                    o_r = out_sb[:].rearrange(
                        "p (hp wp pw) -> p pw hp wp", wp=16, pw=4
                    )
                    nc.scalar.activation(
                        o_r[0:64, :, 8 * t:8 * (t + 1), :], put[:], ACTF.Copy
                    )
                for ph in range(4):
                    eng = nc.scalar if ph == 0 else nc.sync
                    eng.dma_start(
                        out=out[n][:, ph::4, :],
                        in_=out_sb[ph:64:4, :],
                    )

            conv = {}
            conv[0] = emit_load_convert(0)
            emit_rowsq(0, conv[0][2])
            conv[1] = emit_load_convert(1)
            for n in range(NI):
                if n + 2 < NI:
                    conv[n + 2] = emit_load_convert(n + 2)
                x2h, x2l, _ = conv.pop(n)
                emit_scores(n, x2h, x2l)
                if n + 1 < NI:
                    emit_rowsq(n + 1, conv[n + 1][2])
                if n >= 5:
                    emit_unpat(n - 5)
            for m in range(NI - 5, NI):
                emit_unpat(m)

    return nc
_NC_CACHE = None


def _get_nc():
    global _NC_CACHE
    if _NC_CACHE is None:
        _NC_CACHE = build_kernel()
    return _NC_CACHE


def kernel(**inputs: np.ndarray) -> np.ndarray:
    from concourse.bass_utils import run_bass_kernel_spmd

    x = np.ascontiguousarray(inputs["x"], dtype=np.float32)
    cb = np.ascontiguousarray(inputs["codebook"], dtype=np.float32)
    assert x.shape == (128, 16, 64, 64) and cb.shape == (K, D)

    nc = _get_nc()
    in_maps = [
        {"x": x[i * NI:(i + 1) * NI], "codebook": cb} for i in range(N_CORES)
    ]
    res = run_bass_kernel_spmd(nc, in_maps, core_ids=list(range(N_CORES)))
    return np.concatenate([res.results[i]["out"] for i in range(N_CORES)], axis=0)
